# revision 7
# baseline (speedup 1.0000x reference)
"""Expert-parallel MoE (Mixtral-style, low-rank experts) for 8 TRN2 NeuronCores.

Strategy (per sharding hint): shard the E=8 expert axis across the 8 cores.
Host side = the dispatch step: compute router logits/top-2 (fp64 numpy),
gather each expert's routed tokens, transpose to [H, T_pad], and pre-pack
each expert's low-rank factors into matmul-ready transposed stacks.
Device side (Bass/Tile, float32r matmuls): per core e, for its tokens X
  A    = [w1v;v1;w3v;v3] @ X^T                  [636, T]   (stage A)
  g    = [w1u|u1] @ A[0:318];  up = [w3u|u3] @ A[318:636]  (stage B, I-chunked)
  h    = silu(g) * up                                       (stage C)
  c    = [w2v;v2] @ h  (accum over I)           [318, T]   (stage D)
  outT = [w2u|u2] @ c                           [H, T]     (stage E)
Host scatter-adds w_tok * outT^T into the final output.
"""
import math
import numpy as np

H, I, R, E, TOPK = 4096, 14336, 159, 8, 2
R2 = 2 * R        # 318 contraction/width of the paired low-rank factors
ACOLS = 4 * R     # 636 stage-A output rows
NKH = H // 128    # 32
NI = I // 128     # 112
GRP = 8           # I-chunks per stage-D accumulation group

_CHUNKS_R2 = [(0, 128), (128, 128), (256, R2 - 256)]          # contraction 318
_ABLOCKS = [(p * R2 + c0, sz, p, j)                            # (col0, size, path, idx)
            for p in range(2) for j, (c0, sz) in enumerate(_CHUNKS_R2)]


def _install_walrus_wait_fix():
    """The packaged walrus rejects >1 sem wait per instruction
    ('Too many sync wait commands'). After Tile scheduling, hoist extra waits
    onto standalone same-engine nops placed just before the instruction."""
    import concourse.mybir as mybir
    from concourse import tile

    if getattr(tile.TileContext, "_drain_fix_installed", False):
        return

    def _split(nc, cap=1):
        for bb_name in list(nc.bb_map.keys()):
            bbw = nc.bb_map[bb_name]
            bb = bbw.bb if hasattr(bbw, "bb") else bbw
            insts = bb.instructions
            if not any(
                i.sync_info is not None and len(i.sync_info.on_wait) > cap
                for i in insts
            ):
                continue
            new_list = []
            for inst in list(insts):
                si = inst.sync_info
                if si is not None and len(si.on_wait) > cap:
                    waits = list(si.on_wait)
                    si.on_wait = waits[:cap]
                    eng = nc.engines[inst.engine]
                    for j in range(cap, len(waits), cap):
                        nop = eng.nop(nofuse=True, hint="split_waits")
                        cur = nc.cur_bb.bb
                        assert cur.instructions[-1] is nop.ins
                        cur.instructions.pop()
                        nop.ins.sync_info = mybir.SyncInfo(
                            on_wait=waits[j:j + cap], on_update=[])
                        new_list.append(nop.ins)
                new_list.append(inst)
            while bb.instructions:
                bb.instructions.pop()
            for inst in new_list:
                bb.instructions.append(inst)

    orig = tile.TileContext._drain_and_barrier

    def _drain_and_barrier(self, tick_clock, wait_clock):
        orig(self, tick_clock, wait_clock)
        _split(self.nc)

    tile.TileContext._drain_and_barrier = _drain_and_barrier
    tile.TileContext._drain_fix_installed = True


def _t_tiles(T_pad):
    tiles, t0 = [], 0
    while T_pad - t0 >= 512:
        tiles.append((t0, 512))
        t0 += 512
    if T_pad - t0:
        tiles.append((t0, T_pad - t0))
    return tiles


def _build_nc(T_pad):
    import concourse.bass as bass
    import concourse.mybir as mybir
    from concourse import tile

    _install_walrus_wait_fix()
    f32 = mybir.dt.float32
    f32r = mybir.dt.float32r
    Silu = mybir.ActivationFunctionType.Silu
    tt = _t_tiles(T_pad)

    nc = bass.Bass()
    xt_d = nc.declare_dram_parameter("xt", [H, T_pad], f32r, isOutput=False)
    wa_d = nc.declare_dram_parameter("wa_t", [H, ACOLS], f32r, isOutput=False)
    wg_d = nc.declare_dram_parameter("wg_t", [R2, I], f32r, isOutput=False)
    wu_d = nc.declare_dram_parameter("wu_t", [R2, I], f32r, isOutput=False)
    wd_d = nc.declare_dram_parameter("wd_t", [I, R2], f32r, isOutput=False)
    we_d = nc.declare_dram_parameter("we_t", [R2, H], f32r, isOutput=False)
    out_d = nc.declare_dram_parameter("outT", [H, T_pad], f32, isOutput=True)

    def mm(out, lhsT, rhs, start, stop):
        nc.tensor.matmul(out, lhsT.bitcast(f32r), rhs.bitcast(f32r),
                         start=start, stop=stop)

    with tile.TileContext(nc) as tc:
        with (
            tc.tile_pool(name="aout", bufs=1) as apool,
            tc.tile_pool(name="ct", bufs=1) as ctpool,
            tc.tile_pool(name="h", bufs=GRP + 2) as hpool,
        ):
            # stage-A outputs: 2 paths (gate, up) x 3 contraction chunks
            a_sb = [[apool.tile([sz, T_pad], f32r, tag=f"a{p}{j}", name=f"a{p}{j}")
                     for j, (c0, sz) in enumerate(_CHUNKS_R2)]
                    for p in range(2)]
            ct_sb = [ctpool.tile([sz, T_pad], f32r, tag=f"ct{j}", name=f"ct{j}")
                     for j, (c0, sz) in enumerate(_CHUNKS_R2)]

            # ---- stage A ----
            with (
                tc.tile_pool(name="wa", bufs=1) as wapool,
                tc.tile_pool(name="xt", bufs=4) as xtpool,
                tc.tile_pool(name="psA", bufs=7, space="PSUM") as psA,
            ):
                wa_sb = []
                for k in range(NKH):
                    w = wapool.tile([128, ACOLS], f32r, tag=f"wa{k}")
                    nc.sync.dma_start(w[:], wa_d[k * 128:(k + 1) * 128, :])
                    wa_sb.append(w)
                for (t0, tw) in tt:
                    psums = [psA.tile([sz, tw], f32, tag="psA", name=f"psA{bi}")
                             for bi, (c0, sz, p, j) in enumerate(_ABLOCKS)]
                    for k in range(NKH):
                        xtile = xtpool.tile([128, tw], f32r, tag="xt")
                        nc.sync.dma_start(
                            xtile[:], xt_d[k * 128:(k + 1) * 128, t0:t0 + tw])
                        for bi, (c0, sz, p, j) in enumerate(_ABLOCKS):
                            mm(psums[bi][:], wa_sb[k][:, c0:c0 + sz], xtile[:],
                               start=(k == 0), stop=(k == NKH - 1))
                    for bi, (c0, sz, p, j) in enumerate(_ABLOCKS):
                        nc.vector.tensor_copy(a_sb[p][j][:, t0:t0 + tw],
                                              psums[bi][:])

            # ---- stages B/C/D, grouped over I ----
            with (
                tc.tile_pool(name="wbu", bufs=3) as wbpool,
                tc.tile_pool(name="wdp", bufs=GRP + 2) as wdpool,
                tc.tile_pool(name="gs", bufs=3) as gspool,
                tc.tile_pool(name="pgu", bufs=3, space="PSUM") as pgu,
                tc.tile_pool(name="pc", bufs=4, space="PSUM") as pc,
                tc.tile_pool(name="we", bufs=1) as wepool,
                tc.tile_pool(name="oo", bufs=3) as opool,
            ):
                po = pc
                n_grp = NI // GRP
                for grp in range(n_grp):
                    h_tiles = []
                    for ii in range(GRP):
                        ic = grp * GRP + ii
                        wg_sb, wu_sb = [], []
                        for j, (c0, sz) in enumerate(_CHUNKS_R2):
                            wgt = wbpool.tile([sz, 128], f32r, tag=f"wg{j}")
                            nc.sync.dma_start(
                                wgt[:], wg_d[c0:c0 + sz, ic * 128:(ic + 1) * 128])
                            wg_sb.append(wgt)
                            wut = wbpool.tile([sz, 128], f32r, tag=f"wu{j}")
                            nc.sync.dma_start(
                                wut[:], wu_d[c0:c0 + sz, ic * 128:(ic + 1) * 128])
                            wu_sb.append(wut)
                        wdt = wdpool.tile([128, R2], f32r, tag="wd")
                        nc.sync.dma_start(
                            wdt[:], wd_d[ic * 128:(ic + 1) * 128, :])
                        h_t = hpool.tile([128, T_pad], f32r, tag="h")
                        for (t0, tw) in tt:
                            pg = pgu.tile([128, tw], f32, tag="pgu")
                            for j in range(3):
                                mm(pg[:], wg_sb[j][:],
                                   a_sb[0][j][:, t0:t0 + tw],
                                   start=(j == 0), stop=(j == 2))
                            gst = gspool.tile([128, tw], f32, tag="gs")
                            nc.scalar.activation(gst[:], pg[:], Silu)
                            pu = pgu.tile([128, tw], f32, tag="pgu")
                            for j in range(3):
                                mm(pu[:], wu_sb[j][:],
                                   a_sb[1][j][:, t0:t0 + tw],
                                   start=(j == 0), stop=(j == 2))
                            nc.vector.tensor_mul(h_t[:, t0:t0 + tw], gst[:],
                                                 pu[:])
                        h_tiles.append((h_t, wdt))
                    for (t0, tw) in tt:
                        pcs = [pc.tile([sz, tw], f32, tag="pc", name=f"pc{j}")
                               for j, (c0, sz) in enumerate(_CHUNKS_R2)]
                        for ii, (h_t, wdt) in enumerate(h_tiles):
                            for j, (c0, sz) in enumerate(_CHUNKS_R2):
                                mm(pcs[j][:], wdt[:, c0:c0 + sz],
                                   h_t[:, t0:t0 + tw],
                                   start=(ii == 0), stop=(ii == GRP - 1))
                        for j in range(3):
                            if grp == 0:
                                nc.vector.tensor_copy(
                                    ct_sb[j][:, t0:t0 + tw], pcs[j][:])
                            else:
                                nc.vector.tensor_add(
                                    ct_sb[j][:, t0:t0 + tw],
                                    ct_sb[j][:, t0:t0 + tw], pcs[j][:])

                # ---- stage E ----
                we_sb = []
                for j, (c0, sz) in enumerate(_CHUNKS_R2):
                    wet = wepool.tile([sz, H], f32r, tag=f"we{j}")
                    nc.sync.dma_start(wet[:], we_d[c0:c0 + sz, :])
                    we_sb.append(wet)
                for m in range(NKH):
                    o_sb = opool.tile([128, T_pad], f32, tag="o")
                    for (t0, tw) in tt:
                        pot = po.tile([128, tw], f32, tag="pc")
                        for j in range(3):
                            mm(pot[:], we_sb[j][:, m * 128:(m + 1) * 128],
                               ct_sb[j][:, t0:t0 + tw],
                               start=(j == 0), stop=(j == 2))
                        nc.vector.tensor_copy(o_sb[:, t0:t0 + tw], pot[:])
                    nc.sync.dma_start(out_d[m * 128:(m + 1) * 128, :], o_sb[:])
    return nc


_NC_CACHE = {}


def kernel(hidden_states, gate_w, w1u, w1v, w2u, w2v, w3u, w3v,
           u1, v1, u2, v2, u3, v3):
    from concourse.bass_utils import run_bass_kernel_spmd

    hs = np.asarray(hidden_states, dtype=np.float32)
    B, S, Hd = hs.shape
    T = B * S
    x = np.ascontiguousarray(hs.reshape(T, Hd))
    gw = np.asarray(gate_w, dtype=np.float32)

    # ---- router (fp64 host: this IS the dispatch/sharding step) ----
    l64 = x.astype(np.float64) @ gw.T.astype(np.float64)
    router_logits = l64.astype(np.float32)
    mx = l64.max(axis=-1, keepdims=True)
    p = np.exp(l64 - mx)
    p /= p.sum(axis=-1, keepdims=True)
    ar = np.arange(T)
    s0 = p.argmax(axis=-1)
    p_m = p.copy()
    p_m[ar, s0] = -1.0
    s1 = p_m.argmax(axis=-1)
    pw0, pw1 = p[ar, s0], p[ar, s1]
    den = pw0 + pw1
    w0 = (pw0 / den).astype(np.float32)
    w1 = (pw1 / den).astype(np.float32)

    idx = [np.where((s0 == e) | (s1 == e))[0] for e in range(E)]
    wtok = [np.where(s0[idx[e]] == e, w0[idx[e]], w1[idx[e]]) for e in range(E)]
    counts = [len(i) for i in idx]
    T_pad = max(256, int(math.ceil(max(counts) / 256.0)) * 256)

    key = T_pad
    if key not in _NC_CACHE:
        _NC_CACHE[key] = _build_nc(T_pad)
    nc = _NC_CACHE[key]

    arrs = dict(w1u=np.asarray(w1u, np.float32), w1v=np.asarray(w1v, np.float32),
                w2u=np.asarray(w2u, np.float32), w2v=np.asarray(w2v, np.float32),
                w3u=np.asarray(w3u, np.float32), w3v=np.asarray(w3v, np.float32),
                u1=np.asarray(u1, np.float32), v1=np.asarray(v1, np.float32),
                u2=np.asarray(u2, np.float32), v2=np.asarray(v2, np.float32),
                u3=np.asarray(u3, np.float32), v3=np.asarray(v3, np.float32))

    in_maps = []
    for e in range(E):
        xt = np.zeros((Hd, T_pad), np.float32)
        xt[:, :counts[e]] = x[idx[e]].T
        in_maps.append({
            "xt": xt,
            "wa_t": np.ascontiguousarray(np.concatenate(
                [arrs["w1v"][e], arrs["v1"][e], arrs["w3v"][e], arrs["v3"][e]],
                axis=0).T),
            "wg_t": np.ascontiguousarray(np.concatenate(
                [arrs["w1u"][e], arrs["u1"][e]], axis=1).T),
            "wu_t": np.ascontiguousarray(np.concatenate(
                [arrs["w3u"][e], arrs["u3"][e]], axis=1).T),
            "wd_t": np.ascontiguousarray(np.concatenate(
                [arrs["w2v"][e], arrs["v2"][e]], axis=0).T),
            "we_t": np.ascontiguousarray(np.concatenate(
                [arrs["w2u"][e], arrs["u2"][e]], axis=1).T),
        })

    res = run_bass_kernel_spmd(nc, in_maps, list(range(E)))

    final = np.zeros((T, Hd), np.float32)
    for e in range(E):
        o = res.results[e]["outT"][:, :counts[e]].T   # [n_e, H]
        final[idx[e]] += o * wtok[e][:, None]

    return final.reshape(B, S, Hd), router_logits
